# revision 15
# baseline (speedup 1.0000x reference)
"""GAT layer kernel for Trainium2, SPMD over 8 NeuronCores.

Reference computation (per batch b):
  h  = x @ W_lin.T                          [N, O]
  hp = concat(h, prior[None, :])            [N1, O]
  per head: hp_h = hp @ w_head[h]           [N1, O]
  t = tanh(hp_h); s_src = t @ a_src[h]; s_dst = t @ a_dst[h]
  z[i,j] = s_src[i] + s_dst[j]; y = leaky_relu(z, 0.2)
  y[mask_i | mask_j] = -1e18; p = softmax_j(y)
  out_h = p @ hp_h;  out = mean_h(out_h) + bias

Sharding: core c handles batch b=c//2 and heads h in {2*(c%2), 2*(c%2)+1}.

Mask-compaction: masked-j columns get zero attention weight, and masked-i
rows are exactly uniform attention (handled on host via the head's mean
value row vbar, computed on host -- it is linear in the inputs).  So the
device only processes the ~1000 UNMASKED nodes per batch: the host
compacts x to M=1280 padded slots (slot 0 reserved for the prior node,
tail slots padded; pads are forced to zero weight via a -400 sentinel
folded into their d_j), pre-transposes x and W_lin (bf16 -- the PE's
float32r mode rounds operands to bf16 anyway), and scatters the result
back to full [N1, O].  This shrinks the e-matrix work ~4x.

Per core and head the kernel computes the transposed partial output
  outT[h] = sum_j hp_h[j,:] * e[j,i]   in [O, M]    (unnormalized)
and the softmax denominators sums[h][M]; the host divides, scatters,
fixes masked rows with vbar, averages heads, adds bias.

e is generated by two engine routes (tunable per j-chunk), using
exp(lrelu(z)) = max(exp(z), exp(0.2 z)):
  A (ACT):  e1 = Exp(s + d'[j]-bias), e2 = Exp(0.2 s + 0.2 d''[j])
  V (DVE):  rank-1 t1 = E1*f1[j], t2 = E2*f2[j]  (exp(s_i+d_j) =
            exp(s_i)*exp(d_j)); E-rows precomputed once per head
+ a shared DVE tensor_tensor max.  Row-side (i) rounding cancels exactly
in the softmax; only the j side needs fp32-accurate exponents.  e and V
are bf16 so the dominant PE streams run at 1 cycle/column.
"""

import sys

for _p in ("/opt/trn_rl_repo",):
    if _p not in sys.path:
        sys.path.insert(0, _p)

import os as _os

import numpy as np

import concourse.bass as bass
import concourse.tile as tile
from concourse import bacc, mybir

FP = mybir.dt.float32
FR = mybir.dt.float32r
BF = mybir.dt.bfloat16
U8 = mybir.dt.uint8
N, N1, I, O = 2047, 2048, 256, 128
M = 1280          # compacted node slots (>= max unmasked count, 10*128)
NCH = M // 128    # j-chunks
GRPS = [(0, 512), (512, 1024), (1024, M)]  # i-column groups (PSUM banks)
HPC = 2  # heads per core
NCORES = 8
NEG = -400.0    # pad sentinel folded into d_j
DCLAMP = -43.0  # keeps every exp input inside the ACT table (~[-87, 88])
Tanh = mybir.ActivationFunctionType.Tanh
Exp = mybir.ActivationFunctionType.Exp
ALU = mybir.AluOpType

# per-jc e-generation route, A=ACT-heavy, V=DVE rank-1 (see module doc)
ROUTES = _os.environ.get("GAT_ROUTES", "AAVAVAVVAV")
assert len(ROUTES) == NCH and set(ROUTES) <= set("AV")
# engine for the per-head V=hp@wh PSUM->SBUF casts (gpsimd cannot read PSUM)
VCOPY = _os.environ.get("GAT_VCOPY", "SVSVSVSVSV")
assert len(VCOPY) == NCH and set(VCOPY) <= set("SV")


def c128(c):
    return slice(c * 128, (c + 1) * 128)


def _build() -> bass.Bass:
    nc = bacc.Bacc(None, target_bir_lowering=False, debug=False)
    xT_c = nc.dram_tensor("xT_c", [2, 128, M], BF, kind="ExternalInput")
    wlT_c = nc.dram_tensor("wlT_c", [2, 128, 128], BF, kind="ExternalInput")
    prior_b = nc.dram_tensor("prior_b", [O], FP, kind="ExternalInput")
    negm_c = nc.dram_tensor("negm_c", [128, NCH], FP, kind="ExternalInput")
    w_pair = nc.dram_tensor("w_pair", [HPC, O, O], FP, kind="ExternalInput")
    a_src_p = nc.dram_tensor("a_src_p", [HPC, O], FP, kind="ExternalInput")
    a_dst_p = nc.dram_tensor("a_dst_p", [HPC, O], FP, kind="ExternalInput")
    outT = nc.dram_tensor("outT", [HPC, O, M], BF, kind="ExternalOutput")
    sums = nc.dram_tensor("sums", [HPC, M], BF, kind="ExternalOutput")
    sdst_dram = nc.dram_tensor("sdst_scratch", [HPC, M], FP)

    with tile.TileContext(nc) as tc:
        with (
            tc.tile_pool(name="constp", bufs=1) as constp,
            tc.tile_pool(name="bigp", bufs=1) as bigp,
            tc.tile_pool(name="headp", bufs=2) as headp,
            tc.tile_pool(name="scr16", bufs=6) as scr16,
            tc.tile_pool(name="etp", bufs=8) as etp,
            tc.tile_pool(name="outp", bufs=4) as outp,
        ):
            pools = dict(constp=constp, bigp=bigp, headp=headp,
                         scr16=scr16, etp=etp, outp=outp, tc=tc)
            _body(nc, tc, pools,
                  xT_c, wlT_c, prior_b, negm_c, w_pair, a_src_p, a_dst_p,
                  outT, sums, sdst_dram)
    return nc


def _head_prep(nc, pools, h, hpT, w_pair, a_src_p, a_dst_p,
               sdst_dram, consts):
    """Per-head: tT, s2, d-cols + exps, srcb, E-rows, V."""
    headp, pp = pools["headp"], pools["pp"]
    ones_row, negm_cols = consts

    wh = headp.tile([128, 128], FP, tag="wh")
    nc.sync.dma_start(out=wh, in_=w_pair[h])
    acols = headp.tile([128, 2], FP, tag="acols")
    nc.sync.dma_start(out=acols[:, 0:1], in_=a_src_p[h][:, None])
    nc.sync.dma_start(out=acols[:, 1:2], in_=a_dst_p[h][:, None])
    acols_bf = headp.tile([128, 2], BF, tag="acols_bf")
    nc.vector.tensor_copy(acols_bf, acols)
    wh_r = headp.tile([128, 128], FR, tag="wh_r")
    nc.vector.tensor_copy(wh_r, wh)

    # ---- tT = tanh(wh.T @ hpT)  [128(p), M] bf16 ----
    tT = headp.tile([128, M], BF, tag="tT")
    for st, en in GRPS:
        ph = pp.tile([128, 512], FP, tag="tr")
        nc.tensor.matmul(ph[:, :en - st], wh_r, hpT[:, st:en],
                         start=True, stop=True)
        nc.scalar.activation(tT[:, st:en], ph[:, :en - st], Tanh)

    # ---- s2[0]=s_src, s2[1]=s_dst  [2, M] ----
    s2 = headp.tile([2, M], FR, tag="s2")
    for st, en in GRPS:
        ps2 = pp.tile([128, 512], FP, tag="tr")
        nc.tensor.matmul(ps2[:2, :en - st], acols_bf, tT[:, st:en],
                         start=True, stop=True)
        nc.vector.tensor_copy(s2[:, st:en], ps2[:2, :en - st])

    # ---- d_j as columns via DRAM bounce; fold pad mask; exp tables ----
    nc.sync.dma_start(out=sdst_dram[h, :], in_=s2[1:2, :].bitcast(FP))
    sdc = headp.tile([128, NCH], FP, tag="sdc")
    nc.sync.dma_start(out=sdc,
                      in_=sdst_dram[h, :].rearrange("(c p) -> p c", p=128))
    sdcm = headp.tile([128, NCH], FP, tag="sdcm")
    nc.vector.tensor_tensor(sdcm, sdc, negm_cols, op=ALU.add)
    sdc1 = headp.tile([128, NCH], FP, tag="sdc1")
    nc.vector.tensor_scalar_max(sdc1, sdcm, DCLAMP)
    sdc2 = headp.tile([128, NCH], FP, tag="sdc2")
    nc.vector.tensor_scalar(sdc2, sdcm, 0.2, DCLAMP, op0=ALU.mult, op1=ALU.max)
    f1c = headp.tile([128, NCH], FP, tag="f1c")
    nc.scalar.activation(f1c, sdc1, Exp)
    f2c = headp.tile([128, NCH], FP, tag="f2c")
    nc.scalar.activation(f2c, sdc2, Exp)

    # ---- srcb = broadcast of s_src over partitions; E rows ----
    srcb = headp.tile([128, M], FP, tag="srcb")
    E1rb = headp.tile([128, M], BF, tag="E1rb")
    E2rb = headp.tile([128, M], BF, tag="E2rb")
    for st, en in GRPS:
        pb = pp.tile([128, 512], FP, tag="tr")
        nc.tensor.matmul(pb[:, :en - st], ones_row, s2[0:1, st:en],
                         start=True, stop=True)
        nc.scalar.copy(srcb[:, st:en], pb[:, :en - st])
    nc.scalar.activation(E1rb, srcb, Exp)
    nc.scalar.activation(E2rb, srcb, Exp, scale=0.2)

    # ---- V = hp @ wh  [n(p), O] bf16, per 128-chunk ----
    V = headp.tile([128, M], BF, tag="V")
    for t in range(NCH):
        pv = pp.tile([128, 512], FP, tag="tr")
        nc.tensor.matmul(pv[:, :128], hpT[:, c128(t)], wh_r,
                         start=True, stop=True)
        if VCOPY[t] == "S":
            nc.scalar.copy(V[:, c128(t)], pv[:, :128])
        else:
            nc.vector.tensor_copy(V[:, c128(t)], pv[:, :128])

    return dict(tT=tT, s2=s2, sdcm=sdcm, sdc1=sdc1, sdc2=sdc2,
                f1c=f1c, f2c=f2c, srcb=srcb, E1rb=E1rb, E2rb=E2rb, V=V)


def _mains_interleaved(nc, pools, sts, outT, sums, ones_col_bf):
    """Both heads' j-chunk loops interleaved: doubles the PE feed rate so
    the tensor engine stalls less (and keeps its pstate ramped) while the
    DVE/ACT e-generation for the other head proceeds."""
    scr16, etp = pools["scr16"], pools["etp"]
    headp, outp = pools["headp"], pools["outp"]
    pav0, pav1 = pools["pavs"]
    psums = pools["psums"]

    av = [pav0.tile([128, M], FP, tag="av0", name="av0"),
          pav1.tile([128, M], FP, tag="av1", name="av1")]
    sump = [psums.tile([65, 512], FP, tag="sump0", name="sump0"),
            psums.tile([65, 512], FP, tag="sump1", name="sump1")]

    def sum_slot(h, g, width):
        return sump[h][32 * g:32 * g + 1, :width]

    for jc in range(NCH):
        for h in range(HPC):
            st = sts[h]
            srcb, sdc1, sdc2 = st["srcb"], st["sdc1"], st["sdc2"]
            E1rb, E2rb, f1c, f2c, V = (st["E1rb"], st["E2rb"], st["f1c"],
                                       st["f2c"], st["V"])
            route = ROUTES[jc]
            eT = etp.tile([128, M], BF, tag="eT")
            if route == "A":
                # e = max(exp(z), exp(0.2 z)) = exp(lrelu_0.2(z))
                t1 = scr16.tile([128, M], BF, tag="t1")
                nc.scalar.activation(t1, srcb, Exp, bias=sdc1[:, jc:jc + 1])
                t2 = scr16.tile([128, M], BF, tag="t2")
                nc.scalar.activation(t2, srcb, Exp, bias=sdc2[:, jc:jc + 1],
                                     scale=0.2)
            else:
                t1 = scr16.tile([128, M], BF, tag="t1")
                nc.vector.tensor_scalar(t1, E1rb, f1c[:, jc:jc + 1], None,
                                        op0=ALU.mult)
                t2 = scr16.tile([128, M], BF, tag="t2")
                nc.vector.tensor_scalar(t2, E2rb, f2c[:, jc:jc + 1], None,
                                        op0=ALU.mult)
            nc.vector.tensor_tensor(eT, t1, t2, op=ALU.max)
            for g, (gs, ge) in enumerate(GRPS):
                nc.tensor.matmul(av[h][:, gs:ge], V[:, c128(jc)],
                                 eT[:, gs:ge],
                                 start=(jc == 0), stop=(jc == NCH - 1),
                                 skip_group_check=True)
            for g, (gs, ge) in enumerate(GRPS):
                nc.tensor.matmul(sum_slot(h, g, ge - gs), ones_col_bf,
                                 eT[:, gs:ge],
                                 start=(jc == 0), stop=(jc == NCH - 1),
                                 skip_group_check=True)

    # ---- exports (alternate DVE/ACT so the tail drains on two engines) --
    for h in range(HPC):
        sum_sb = headp.tile([1, M], BF, tag="sum_sb")
        for g, (gs, ge) in enumerate(GRPS):
            nc.vector.tensor_copy(sum_sb[:, gs:ge], sum_slot(h, g, ge - gs))
        nc.sync.dma_start(out=sums[h, :], in_=sum_sb)
        for g, (gs, ge) in enumerate(GRPS):
            outF = outp.tile([128, 512], BF, tag="outF")
            if g % 2 == 0:
                nc.scalar.copy(outF[:, :ge - gs], av[h][:, gs:ge])
            else:
                nc.vector.tensor_copy(outF[:, :ge - gs], av[h][:, gs:ge])
            nc.sync.dma_start(out=outT[h, :, gs:ge], in_=outF[:, :ge - gs])


def _body(nc, tc, pools,
          xT_c, wlT_c, prior_b, negm_c, w_pair, a_src_p, a_dst_p,
          outT, sums, sdst_dram):
    constp, bigp = pools["constp"], pools["bigp"]
    tcx = pools["tc"]
    prep_pool_cm = tcx.tile_pool(name="pp", bufs=3, space="PSUM")
    pp = prep_pool_cm.__enter__()
    pools["pp"] = pp

    # ---- constants ----
    ones_row_f = constp.tile([1, 128], FP, tag="ones_row_f")
    nc.vector.memset(ones_row_f, 1.0)
    ones_row = constp.tile([1, 128], FR, tag="ones_row")
    nc.vector.tensor_copy(ones_row, ones_row_f)
    ones_col_bf = constp.tile([128, 1], BF, tag="ones_col_bf")
    nc.vector.memset(ones_col_bf, 1.0)
    negm_cols = constp.tile([128, NCH], FP, tag="negm_cols")
    nc.sync.dma_start(out=negm_cols, in_=negm_c[:, :])

    # ---- prep: hpT = (x_c @ W_lin.T).T from host-transposed bf16 inputs --
    hpT = bigp.tile([128, M], FR, tag="hpT")
    wlT = constp.tile([128, 2, 128], BF, tag="wlT")
    xT = bigp.tile([128, 2, M], BF, tag="xT")
    prior_sb = constp.tile([128, 1], FP, tag="prior_sb")
    nc.sync.dma_start(out=prior_sb, in_=prior_b[:, None])
    for k in range(2):
        nc.sync.dma_start(out=wlT[:, k, :], in_=wlT_c[k])
        for st, en in GRPS:
            nc.sync.dma_start(out=xT[:, k, st:en], in_=xT_c[k][:, st:en])
    for st, en in GRPS:
        ph = pp.tile([128, 512], FP, tag="tr")
        for k in range(2):
            nc.tensor.matmul(ph[:, :en - st], wlT[:, k, :], xT[:, k, st:en],
                             start=(k == 0), stop=(k == 1))
        nc.vector.tensor_copy(hpT[:, st:en], ph[:, :en - st])
    # slot 0 is reserved for the prior node
    nc.vector.tensor_copy(hpT[:, 0:1], prior_sb)

    consts_prep = (ones_row, negm_cols)
    sts = []
    for h in range(HPC):
        sts.append(_head_prep(nc, pools, h, hpT,
                              w_pair, a_src_p, a_dst_p,
                              sdst_dram, consts_prep))
    prep_pool_cm.__exit__(None, None, None)
    with (
        tcx.tile_pool(name="pav0", bufs=1, space="PSUM") as pav0,
        tcx.tile_pool(name="pav1", bufs=1, space="PSUM") as pav1,
        tcx.tile_pool(name="psums", bufs=1, space="PSUM") as psums,
    ):
        pools["pavs"] = (pav0, pav1)
        pools["psums"] = psums
        _mains_interleaved(nc, pools, sts, outT, sums, ones_col_bf)


_NC_CACHE = None


def _get_nc():
    global _NC_CACHE
    if _NC_CACHE is None:
        nc = _build()
        nc.finalize()
        _NC_CACHE = nc
    return _NC_CACHE


def _compact(x, x_mask):
    """Per batch: slot 0 = prior node (2047), then unmasked nodes, then pads.

    Returns per-batch (xT_c bf16 [2,128,M], negm_c fp32 [M],
    idx array of real node ids for slots 1.., n_real, prior_keep).
    """
    import ml_dtypes
    B = x.shape[0]
    packs = []
    for b in range(B):
        keep = ~x_mask[b]
        others = np.nonzero(keep[:N])[0]
        n_real = 1 + len(others)
        assert n_real <= M, f"batch {b}: {n_real} unmasked nodes > M={M}"
        xc = np.zeros((M, I), np.float32)
        xc[1:n_real] = x[b][others]
        negm = np.zeros(M, np.float32)
        negm[n_real:] = NEG
        if not keep[N]:          # prior node masked -> slot 0 is a pad
            negm[0] = NEG
        negm = np.ascontiguousarray(negm.reshape(NCH, 128).T)
        xT = np.ascontiguousarray(
            xc.T.reshape(2, 128, M).astype(ml_dtypes.bfloat16))
        packs.append((xT, negm, others, n_real, bool(keep[N])))
    return packs


def make_in_maps(x, prior_feature, x_mask, W_lin, w_head, a_src, a_dst):
    import ml_dtypes
    packs = _compact(x, x_mask)
    wlT_c = np.ascontiguousarray(
        W_lin.T.reshape(2, 128, 128).astype(ml_dtypes.bfloat16))
    in_maps = []
    for c in range(NCORES):
        b, h0 = c // 2, (c % 2) * HPC
        xT, negm, _, _, _ = packs[b]
        in_maps.append(dict(
            xT_c=xT,
            wlT_c=wlT_c,
            prior_b=prior_feature[b],
            negm_c=negm,
            w_pair=np.ascontiguousarray(w_head[h0:h0 + HPC]),
            a_src_p=np.ascontiguousarray(a_src[h0:h0 + HPC]),
            a_dst_p=np.ascontiguousarray(a_dst[h0:h0 + HPC]),
        ))
    return packs, in_maps


def combine_results(results, packs, x, prior_feature, x_mask,
                    W_lin, w_head, bias):
    B = 4
    out = np.zeros((B, N1, O), np.float32)
    for c in range(NCORES):
        b = c // 2
        o = np.asarray(results[c]["outT"], np.float32)   # [HPC, O, M]
        s = np.asarray(results[c]["sums"], np.float32)    # [HPC, M]
        _, _, others, n_real, prior_keep = packs[b]
        contrib = ((o[0] / s[0][None, :] + o[1] / s[1][None, :]).T
                   * 0.25)[:n_real]
        if prior_keep:
            out[b, N] += contrib[0]
        out[b, others] += contrib[1:]
    # masked rows: exactly uniform attention = mean_j hp_h[j] (host, exact)
    xsum = x.sum(axis=1)                                   # [B, I]
    hp_mean = (xsum @ W_lin.T + prior_feature) / N1        # [B, O]
    vbar_sum = np.einsum('bo,hop->bp', hp_mean, w_head)    # sum over heads
    for b in range(B):
        out[b][x_mask[b], :] = 0.25 * vbar_sum[b][None, :]
    out += np.asarray(bias, np.float32)[None, None, :]
    return out


def kernel(x, prior_feature, x_mask, W_lin, w_head, a_src, a_dst, bias,
           **run_kwargs):
    from concourse.bass_utils import run_bass_kernel_spmd
    nc = _get_nc()
    x = np.ascontiguousarray(np.asarray(x, np.float32))
    prior_feature = np.ascontiguousarray(np.asarray(prior_feature, np.float32))
    x_mask = np.asarray(x_mask, bool)
    W_lin = np.ascontiguousarray(np.asarray(W_lin, np.float32))
    w_head = np.ascontiguousarray(np.asarray(w_head, np.float32))
    a_src = np.ascontiguousarray(np.asarray(a_src, np.float32))
    a_dst = np.ascontiguousarray(np.asarray(a_dst, np.float32))
    packs, in_maps = make_in_maps(x, prior_feature, x_mask, W_lin, w_head,
                                  a_src, a_dst)
    br = run_bass_kernel_spmd(nc, in_maps, core_ids=list(range(NCORES)),
                              **run_kwargs)
    out = combine_results(br.results, packs, x, prior_feature, x_mask,
                          W_lin, w_head, bias)
    if run_kwargs:
        kernel.last_bass_results = br
    return out


# revision 16
# speedup vs baseline: 1.0365x; 1.0365x over previous
"""GAT layer kernel for Trainium2, SPMD over 8 NeuronCores.

Reference computation (per batch b):
  h  = x @ W_lin.T                          [N, O]
  hp = concat(h, prior[None, :])            [N1, O]
  per head: hp_h = hp @ w_head[h]           [N1, O]
  t = tanh(hp_h); s_src = t @ a_src[h]; s_dst = t @ a_dst[h]
  z[i,j] = s_src[i] + s_dst[j]; y = leaky_relu(z, 0.2)
  y[mask_i | mask_j] = -1e18; p = softmax_j(y)
  out_h = p @ hp_h;  out = mean_h(out_h) + bias

Sharding: core c handles batch b=c//2 and heads h in {2*(c%2), 2*(c%2)+1}.

Mask-compaction: masked-j columns get zero attention weight, and masked-i
rows are exactly uniform attention (handled on host via the head's mean
value row vbar, computed on host -- it is linear in the inputs).  So the
device only processes the ~1000 UNMASKED nodes per batch: the host
compacts x to M=1280 padded slots (slot 0 reserved for the prior node,
tail slots padded; pads are forced to zero weight via a -400 sentinel
folded into their d_j), pre-transposes x and W_lin (bf16 -- the PE's
float32r mode rounds operands to bf16 anyway), and scatters the result
back to full [N1, O].  This shrinks the e-matrix work ~4x.

Per core and head the kernel computes the transposed partial output
  outT[h] = sum_j hp_h[j,:] * e[j,i]   in [O, M]    (unnormalized)
and the softmax denominators sums[h][M]; the host divides, scatters,
fixes masked rows with vbar, averages heads, adds bias.

e is generated by two engine routes (tunable per j-chunk), using
exp(lrelu(z)) = max(exp(z), exp(0.2 z)):
  A (ACT):  e1 = Exp(s + d'[j]-bias), e2 = Exp(0.2 s + 0.2 d''[j])
  V (DVE):  rank-1 t1 = E1*f1[j], t2 = E2*f2[j]  (exp(s_i+d_j) =
            exp(s_i)*exp(d_j)); E-rows precomputed once per head
+ a shared DVE tensor_tensor max.  Row-side (i) rounding cancels exactly
in the softmax; only the j side needs fp32-accurate exponents.  e and V
are bf16 so the dominant PE streams run at 1 cycle/column.
"""

import sys

for _p in ("/opt/trn_rl_repo",):
    if _p not in sys.path:
        sys.path.insert(0, _p)

import os as _os

import numpy as np

import concourse.bass as bass
import concourse.tile as tile
from concourse import bacc, mybir

FP = mybir.dt.float32
FR = mybir.dt.float32r
BF = mybir.dt.bfloat16
U8 = mybir.dt.uint8
N, N1, I, O = 2047, 2048, 256, 128
M = 1280          # compacted node slots (>= max unmasked count, 10*128)
NCH = M // 128    # j-chunks
GRPS = [(0, 512), (512, 1024), (1024, M)]  # i-column groups (PSUM banks)
HPC = 2  # heads per core
NCORES = 8
NEG = -400.0    # pad sentinel folded into d_j
DCLAMP = -43.0  # keeps every exp input inside the ACT table (~[-87, 88])
Tanh = mybir.ActivationFunctionType.Tanh
Exp = mybir.ActivationFunctionType.Exp
ALU = mybir.AluOpType

# per-jc e-generation route, A=ACT-heavy, V=DVE rank-1 (see module doc)
ROUTES = _os.environ.get("GAT_ROUTES", "AAVAVAVVAV")
assert len(ROUTES) == NCH and set(ROUTES) <= set("AV")
# engine for the per-head V=hp@wh PSUM->SBUF casts (gpsimd cannot read PSUM)
VCOPY = _os.environ.get("GAT_VCOPY", "SVSVSVSVSV")
assert len(VCOPY) == NCH and set(VCOPY) <= set("SV")


def c128(c):
    return slice(c * 128, (c + 1) * 128)


def _build() -> bass.Bass:
    nc = bacc.Bacc(None, target_bir_lowering=False, debug=False)
    xT_c = nc.dram_tensor("xT_c", [2, 128, M], BF, kind="ExternalInput")
    wlT_c = nc.dram_tensor("wlT_c", [2, 128, 128], BF, kind="ExternalInput")
    prior_b = nc.dram_tensor("prior_b", [O], FP, kind="ExternalInput")
    negm_c = nc.dram_tensor("negm_c", [128, NCH], FP, kind="ExternalInput")
    w_pair = nc.dram_tensor("w_pair", [HPC, O, O], FP, kind="ExternalInput")
    a_src_p = nc.dram_tensor("a_src_p", [HPC, O], FP, kind="ExternalInput")
    a_dst_p = nc.dram_tensor("a_dst_p", [HPC, O], FP, kind="ExternalInput")
    outT = nc.dram_tensor("outT", [HPC, O, M], BF, kind="ExternalOutput")
    sums = nc.dram_tensor("sums", [HPC, M], BF, kind="ExternalOutput")
    sdst_dram = nc.dram_tensor("sdst_scratch", [HPC, M], FP)

    with tile.TileContext(nc) as tc:
        with (
            tc.tile_pool(name="constp", bufs=1) as constp,
            tc.tile_pool(name="bigp", bufs=1) as bigp,
            tc.tile_pool(name="headp", bufs=2) as headp,
            tc.tile_pool(name="scr16", bufs=6) as scr16,
            tc.tile_pool(name="etp", bufs=8) as etp,
            tc.tile_pool(name="outp", bufs=4) as outp,
            tc.tile_pool(name="pp", bufs=3, space="PSUM") as pp,
            tc.tile_pool(name="pav", bufs=1, space="PSUM") as pav,
            tc.tile_pool(name="psums", bufs=1, space="PSUM") as psums,
        ):
            pools = dict(constp=constp, bigp=bigp, headp=headp,
                         scr16=scr16, etp=etp, outp=outp,
                         pp=pp, pav=pav, psums=psums, tc=tc)
            _body(nc, tc, pools,
                  xT_c, wlT_c, prior_b, negm_c, w_pair, a_src_p, a_dst_p,
                  outT, sums, sdst_dram)
    return nc


def _head_prep(nc, pools, h, hpT, w_pair, a_src_p, a_dst_p,
               sdst_dram, consts):
    """Per-head: tT, s2, d-cols + exps, srcb, E-rows, V."""
    headp, pp = pools["headp"], pools["pp"]
    ones_row, negm_cols = consts

    wh = headp.tile([128, 128], FP, tag="wh")
    nc.sync.dma_start(out=wh, in_=w_pair[h])
    acols = headp.tile([128, 2], FP, tag="acols")
    nc.sync.dma_start(out=acols[:, 0:1], in_=a_src_p[h][:, None])
    nc.sync.dma_start(out=acols[:, 1:2], in_=a_dst_p[h][:, None])
    acols_bf = headp.tile([128, 2], BF, tag="acols_bf")
    nc.vector.tensor_copy(acols_bf, acols)
    wh_r = headp.tile([128, 128], FR, tag="wh_r")
    nc.vector.tensor_copy(wh_r, wh)

    # ---- tT = tanh(wh.T @ hpT)  [128(p), M] bf16 ----
    tT = headp.tile([128, M], BF, tag="tT")
    for st, en in GRPS:
        ph = pp.tile([128, 512], FP, tag="tr")
        nc.tensor.matmul(ph[:, :en - st], wh_r, hpT[:, st:en],
                         start=True, stop=True)
        nc.scalar.activation(tT[:, st:en], ph[:, :en - st], Tanh)

    # ---- s2[0]=s_src, s2[1]=s_dst  [2, M] ----
    s2 = headp.tile([2, M], FR, tag="s2")
    for st, en in GRPS:
        ps2 = pp.tile([128, 512], FP, tag="tr")
        nc.tensor.matmul(ps2[:2, :en - st], acols_bf, tT[:, st:en],
                         start=True, stop=True)
        nc.vector.tensor_copy(s2[:, st:en], ps2[:2, :en - st])

    # ---- d_j as columns via DRAM bounce; fold pad mask; exp tables ----
    nc.sync.dma_start(out=sdst_dram[h, :], in_=s2[1:2, :].bitcast(FP))
    sdc = headp.tile([128, NCH], FP, tag="sdc")
    nc.sync.dma_start(out=sdc,
                      in_=sdst_dram[h, :].rearrange("(c p) -> p c", p=128))
    sdcm = headp.tile([128, NCH], FP, tag="sdcm")
    nc.vector.tensor_tensor(sdcm, sdc, negm_cols, op=ALU.add)
    sdc1 = headp.tile([128, NCH], FP, tag="sdc1")
    nc.vector.tensor_scalar_max(sdc1, sdcm, DCLAMP)
    sdc2 = headp.tile([128, NCH], FP, tag="sdc2")
    nc.vector.tensor_scalar(sdc2, sdcm, 0.2, DCLAMP, op0=ALU.mult, op1=ALU.max)
    f1c = headp.tile([128, NCH], FP, tag="f1c")
    nc.scalar.activation(f1c, sdc1, Exp)
    f2c = headp.tile([128, NCH], FP, tag="f2c")
    nc.scalar.activation(f2c, sdc2, Exp)

    # ---- srcb = broadcast of s_src over partitions; E rows ----
    srcb = headp.tile([128, M], FP, tag="srcb")
    E1rb = headp.tile([128, M], BF, tag="E1rb")
    E2rb = headp.tile([128, M], BF, tag="E2rb")
    for st, en in GRPS:
        pb = pp.tile([128, 512], FP, tag="tr")
        nc.tensor.matmul(pb[:, :en - st], ones_row, s2[0:1, st:en],
                         start=True, stop=True)
        nc.scalar.copy(srcb[:, st:en], pb[:, :en - st])
    nc.scalar.activation(E1rb, srcb, Exp)
    nc.scalar.activation(E2rb, srcb, Exp, scale=0.2)

    # ---- V = hp @ wh  [n(p), O] bf16, per 128-chunk ----
    V = headp.tile([128, M], BF, tag="V")
    for t in range(NCH):
        pv = pp.tile([128, 512], FP, tag="tr")
        nc.tensor.matmul(pv[:, :128], hpT[:, c128(t)], wh_r,
                         start=True, stop=True)
        if VCOPY[t] == "S":
            nc.scalar.copy(V[:, c128(t)], pv[:, :128])
        else:
            nc.vector.tensor_copy(V[:, c128(t)], pv[:, :128])

    return dict(tT=tT, s2=s2, sdcm=sdcm, sdc1=sdc1, sdc2=sdc2,
                f1c=f1c, f2c=f2c, srcb=srcb, E1rb=E1rb, E2rb=E2rb, V=V)


def _head_main(nc, pools, h, st, outT, sums, consts):
    scr16, etp = pools["scr16"], pools["etp"]
    headp, outp = pools["headp"], pools["outp"]
    pav, psums = pools["pav"], pools["psums"]
    ones_col_bf = consts

    srcb, sdc1, sdc2 = st["srcb"], st["sdc1"], st["sdc2"]
    E1rb, E2rb, f1c, f2c, V = st["E1rb"], st["E2rb"], st["f1c"], st["f2c"], st["V"]

    av = pav.tile([128, M], FP, tag="av")
    sump = psums.tile([65, 512], FP, tag="sump")

    def sum_slot(g, width):
        base = 32 * g
        return sump[base:base + 1, :width]

    for jc in range(NCH):
        route = ROUTES[jc]
        eT = etp.tile([128, M], BF, tag="eT")
        if route == "A":
            # e = max(exp(z), exp(0.2 z)) = exp(lrelu_0.2(z)), z = s_i + d_j
            t1 = scr16.tile([128, M], BF, tag="t1")
            nc.scalar.activation(t1, srcb, Exp, bias=sdc1[:, jc:jc + 1])
            t2 = scr16.tile([128, M], BF, tag="t2")
            nc.scalar.activation(t2, srcb, Exp, bias=sdc2[:, jc:jc + 1],
                                 scale=0.2)
        else:
            t1 = scr16.tile([128, M], BF, tag="t1")
            nc.vector.tensor_scalar(t1, E1rb, f1c[:, jc:jc + 1], None,
                                    op0=ALU.mult)
            t2 = scr16.tile([128, M], BF, tag="t2")
            nc.vector.tensor_scalar(t2, E2rb, f2c[:, jc:jc + 1], None,
                                    op0=ALU.mult)
        nc.vector.tensor_tensor(eT, t1, t2, op=ALU.max)
        for g, (gs, ge) in enumerate(GRPS):
            nc.tensor.matmul(av[:, gs:ge], V[:, c128(jc)], eT[:, gs:ge],
                             start=(jc == 0), stop=(jc == NCH - 1),
                             skip_group_check=True)
        for g, (gs, ge) in enumerate(GRPS):
            nc.tensor.matmul(sum_slot(g, ge - gs), ones_col_bf, eT[:, gs:ge],
                             start=(jc == 0), stop=(jc == NCH - 1),
                             skip_group_check=True)

    # ---- export unnormalized av + denominators; host divides ----
    sum_sb = headp.tile([1, M], BF, tag="sum_sb")
    for g, (gs, ge) in enumerate(GRPS):
        nc.vector.tensor_copy(sum_sb[:, gs:ge], sum_slot(g, ge - gs))
    nc.sync.dma_start(out=sums[h, :], in_=sum_sb)
    for gs, ge in GRPS:
        outF = outp.tile([128, 512], BF, tag="outF")
        nc.vector.tensor_copy(outF[:, :ge - gs], av[:, gs:ge])
        nc.sync.dma_start(out=outT[h, :, gs:ge], in_=outF[:, :ge - gs])


def _body(nc, tc, pools,
          xT_c, wlT_c, prior_b, negm_c, w_pair, a_src_p, a_dst_p,
          outT, sums, sdst_dram):
    constp, bigp = pools["constp"], pools["bigp"]
    pp = pools["pp"]

    # ---- constants ----
    ones_row_f = constp.tile([1, 128], FP, tag="ones_row_f")
    nc.vector.memset(ones_row_f, 1.0)
    ones_row = constp.tile([1, 128], FR, tag="ones_row")
    nc.vector.tensor_copy(ones_row, ones_row_f)
    ones_col_bf = constp.tile([128, 1], BF, tag="ones_col_bf")
    nc.vector.memset(ones_col_bf, 1.0)
    negm_cols = constp.tile([128, NCH], FP, tag="negm_cols")
    nc.sync.dma_start(out=negm_cols, in_=negm_c[:, :])

    # ---- prep: hpT = (x_c @ W_lin.T).T from host-transposed bf16 inputs --
    hpT = bigp.tile([128, M], FR, tag="hpT")
    wlT = constp.tile([128, 2, 128], BF, tag="wlT")
    xT = bigp.tile([128, 2, M], BF, tag="xT")
    prior_sb = constp.tile([128, 1], FP, tag="prior_sb")
    nc.sync.dma_start(out=prior_sb, in_=prior_b[:, None])
    for k in range(2):
        nc.sync.dma_start(out=wlT[:, k, :], in_=wlT_c[k])
        nc.sync.dma_start(out=xT[:, k, :], in_=xT_c[k])
    for st, en in GRPS:
        ph = pp.tile([128, 512], FP, tag="tr")
        for k in range(2):
            nc.tensor.matmul(ph[:, :en - st], wlT[:, k, :], xT[:, k, st:en],
                             start=(k == 0), stop=(k == 1))
        nc.vector.tensor_copy(hpT[:, st:en], ph[:, :en - st])
    # slot 0 is reserved for the prior node
    nc.vector.tensor_copy(hpT[:, 0:1], prior_sb)

    consts_prep = (ones_row, negm_cols)
    sts = []
    for h in range(HPC):
        sts.append(_head_prep(nc, pools, h, hpT,
                              w_pair, a_src_p, a_dst_p,
                              sdst_dram, consts_prep))
    for h in range(HPC):
        _head_main(nc, pools, h, sts[h], outT, sums, ones_col_bf)


_NC_CACHE = None


def _get_nc():
    global _NC_CACHE
    if _NC_CACHE is None:
        nc = _build()
        nc.finalize()
        _NC_CACHE = nc
    return _NC_CACHE


def _compact(x, x_mask):
    """Per batch: slot 0 = prior node (2047), then unmasked nodes, then pads.

    Returns per-batch (xT_c bf16 [2,128,M], negm_c fp32 [M],
    idx array of real node ids for slots 1.., n_real, prior_keep).
    """
    import ml_dtypes
    B = x.shape[0]
    packs = []
    for b in range(B):
        keep = ~x_mask[b]
        others = np.nonzero(keep[:N])[0]
        n_real = 1 + len(others)
        assert n_real <= M, f"batch {b}: {n_real} unmasked nodes > M={M}"
        xc = np.zeros((M, I), np.float32)
        xc[1:n_real] = x[b][others]
        negm = np.zeros(M, np.float32)
        negm[n_real:] = NEG
        if not keep[N]:          # prior node masked -> slot 0 is a pad
            negm[0] = NEG
        negm = np.ascontiguousarray(negm.reshape(NCH, 128).T)
        xT = np.ascontiguousarray(
            xc.T.reshape(2, 128, M).astype(ml_dtypes.bfloat16))
        packs.append((xT, negm, others, n_real, bool(keep[N])))
    return packs


def make_in_maps(x, prior_feature, x_mask, W_lin, w_head, a_src, a_dst):
    import ml_dtypes
    packs = _compact(x, x_mask)
    wlT_c = np.ascontiguousarray(
        W_lin.T.reshape(2, 128, 128).astype(ml_dtypes.bfloat16))
    in_maps = []
    for c in range(NCORES):
        b, h0 = c // 2, (c % 2) * HPC
        xT, negm, _, _, _ = packs[b]
        in_maps.append(dict(
            xT_c=xT,
            wlT_c=wlT_c,
            prior_b=prior_feature[b],
            negm_c=negm,
            w_pair=np.ascontiguousarray(w_head[h0:h0 + HPC]),
            a_src_p=np.ascontiguousarray(a_src[h0:h0 + HPC]),
            a_dst_p=np.ascontiguousarray(a_dst[h0:h0 + HPC]),
        ))
    return packs, in_maps


def combine_results(results, packs, x, prior_feature, x_mask,
                    W_lin, w_head, bias):
    B = 4
    out = np.zeros((B, N1, O), np.float32)
    for c in range(NCORES):
        b = c // 2
        o = np.asarray(results[c]["outT"], np.float32)   # [HPC, O, M]
        s = np.asarray(results[c]["sums"], np.float32)    # [HPC, M]
        _, _, others, n_real, prior_keep = packs[b]
        contrib = ((o[0] / s[0][None, :] + o[1] / s[1][None, :]).T
                   * 0.25)[:n_real]
        if prior_keep:
            out[b, N] += contrib[0]
        out[b, others] += contrib[1:]
    # masked rows: exactly uniform attention = mean_j hp_h[j] (host, exact)
    xsum = x.sum(axis=1)                                   # [B, I]
    hp_mean = (xsum @ W_lin.T + prior_feature) / N1        # [B, O]
    vbar_sum = np.einsum('bo,hop->bp', hp_mean, w_head)    # sum over heads
    for b in range(B):
        out[b][x_mask[b], :] = 0.25 * vbar_sum[b][None, :]
    out += np.asarray(bias, np.float32)[None, None, :]
    return out


def kernel(x, prior_feature, x_mask, W_lin, w_head, a_src, a_dst, bias,
           **run_kwargs):
    from concourse.bass_utils import run_bass_kernel_spmd
    nc = _get_nc()
    x = np.ascontiguousarray(np.asarray(x, np.float32))
    prior_feature = np.ascontiguousarray(np.asarray(prior_feature, np.float32))
    x_mask = np.asarray(x_mask, bool)
    W_lin = np.ascontiguousarray(np.asarray(W_lin, np.float32))
    w_head = np.ascontiguousarray(np.asarray(w_head, np.float32))
    a_src = np.ascontiguousarray(np.asarray(a_src, np.float32))
    a_dst = np.ascontiguousarray(np.asarray(a_dst, np.float32))
    packs, in_maps = make_in_maps(x, prior_feature, x_mask, W_lin, w_head,
                                  a_src, a_dst)
    br = run_bass_kernel_spmd(nc, in_maps, core_ids=list(range(NCORES)),
                              **run_kwargs)
    out = combine_results(br.results, packs, x, prior_feature, x_mask,
                          W_lin, w_head, bias)
    if run_kwargs:
        kernel.last_bass_results = br
    return out


# revision 17
# speedup vs baseline: 1.1227x; 1.0832x over previous
"""GAT layer kernel for Trainium2, SPMD over 8 NeuronCores.

Reference computation (per batch b):
  h  = x @ W_lin.T                          [N, O]
  hp = concat(h, prior[None, :])            [N1, O]
  per head: hp_h = hp @ w_head[h]           [N1, O]
  t = tanh(hp_h); s_src = t @ a_src[h]; s_dst = t @ a_dst[h]
  z[i,j] = s_src[i] + s_dst[j]; y = leaky_relu(z, 0.2)
  y[mask_i | mask_j] = -1e18; p = softmax_j(y)
  out_h = p @ hp_h;  out = mean_h(out_h) + bias

Sharding: core c handles batch b=c//2 and heads h in {2*(c%2), 2*(c%2)+1}.

Mask-compaction: masked-j columns get zero attention weight, and masked-i
rows are exactly uniform attention (handled on host via the head's mean
value row vbar, computed on host -- it is linear in the inputs).  So the
device only processes the ~1000 UNMASKED nodes per batch: the host
compacts x to M=1280 padded slots (slot 0 reserved for the prior node,
tail slots padded; pads are forced to zero weight via a -400 sentinel
folded into their d_j), pre-transposes x and W_lin (bf16 -- the PE's
float32r mode rounds operands to bf16 anyway), and scatters the result
back to full [N1, O].  This shrinks the e-matrix work ~4x.

Per core and head the kernel computes the transposed partial output
  outT[h] = sum_j hp_h[j,:] * e[j,i]   in [O, M]    (unnormalized)
and the softmax denominators sums[h][M]; the host divides, scatters,
fixes masked rows with vbar, averages heads, adds bias.

e is generated by two engine routes (tunable per j-chunk), using
exp(lrelu(z)) = max(exp(z), exp(0.2 z)):
  A (ACT):  e1 = Exp(s + d'[j]-bias), e2 = Exp(0.2 s + 0.2 d''[j])
  V (DVE):  rank-1 t1 = E1*f1[j], t2 = E2*f2[j]  (exp(s_i+d_j) =
            exp(s_i)*exp(d_j)); E-rows precomputed once per head
+ a shared DVE tensor_tensor max.  Row-side (i) rounding cancels exactly
in the softmax; only the j side needs fp32-accurate exponents.  e and V
are bf16 so the dominant PE streams run at 1 cycle/column.
"""

import sys

for _p in ("/opt/trn_rl_repo",):
    if _p not in sys.path:
        sys.path.insert(0, _p)

import os as _os

import numpy as np

import concourse.bass as bass
import concourse.tile as tile
from concourse import bacc, mybir

FP = mybir.dt.float32
FR = mybir.dt.float32r
BF = mybir.dt.bfloat16
U8 = mybir.dt.uint8
N, N1, I, O = 2047, 2048, 256, 128
M = 1152          # compacted node slots (>= max unmasked count, 9*128)
NCH = M // 128    # j-chunks
GRPS = [(0, 512), (512, 1024), (1024, M)]  # i-column groups (PSUM banks)
HPC = 2  # heads per core
NCORES = 8
NEG = -400.0    # pad sentinel folded into d_j
DCLAMP = -43.0  # keeps every exp input inside the ACT table (~[-87, 88])
Tanh = mybir.ActivationFunctionType.Tanh
Exp = mybir.ActivationFunctionType.Exp
ALU = mybir.AluOpType

# per-jc e-generation route, A=ACT-heavy, V=DVE rank-1 (see module doc)
ROUTES = _os.environ.get("GAT_ROUTES", "AAVAVAVVA")
assert len(ROUTES) == NCH and set(ROUTES) <= set("AV")
# engine for the per-head V=hp@wh PSUM->SBUF casts (gpsimd cannot read PSUM)
VCOPY = _os.environ.get("GAT_VCOPY", "SVSVSVSVS")
assert len(VCOPY) == NCH and set(VCOPY) <= set("SV")


def c128(c):
    return slice(c * 128, (c + 1) * 128)


def _build() -> bass.Bass:
    nc = bacc.Bacc(None, target_bir_lowering=False, debug=False)
    xT_c = nc.dram_tensor("xT_c", [2, 128, M], BF, kind="ExternalInput")
    wlT_c = nc.dram_tensor("wlT_c", [2, 128, 128], BF, kind="ExternalInput")
    prior_b = nc.dram_tensor("prior_b", [O], FP, kind="ExternalInput")
    negm_c = nc.dram_tensor("negm_c", [128, NCH], FP, kind="ExternalInput")
    w_pair = nc.dram_tensor("w_pair", [HPC, O, O], FP, kind="ExternalInput")
    a_src_p = nc.dram_tensor("a_src_p", [HPC, O], FP, kind="ExternalInput")
    a_dst_p = nc.dram_tensor("a_dst_p", [HPC, O], FP, kind="ExternalInput")
    outT = nc.dram_tensor("outT", [HPC, O, M], BF, kind="ExternalOutput")
    sums = nc.dram_tensor("sums", [HPC, M], BF, kind="ExternalOutput")
    sdst_dram = nc.dram_tensor("sdst_scratch", [HPC, M], FP)

    with tile.TileContext(nc) as tc:
        with (
            tc.tile_pool(name="constp", bufs=1) as constp,
            tc.tile_pool(name="bigp", bufs=1) as bigp,
            tc.tile_pool(name="headp", bufs=2) as headp,
            tc.tile_pool(name="scr16", bufs=6) as scr16,
            tc.tile_pool(name="etp", bufs=8) as etp,
            tc.tile_pool(name="outp", bufs=4) as outp,
            tc.tile_pool(name="pp", bufs=3, space="PSUM") as pp,
            tc.tile_pool(name="pav", bufs=1, space="PSUM") as pav,
            tc.tile_pool(name="psums", bufs=1, space="PSUM") as psums,
        ):
            pools = dict(constp=constp, bigp=bigp, headp=headp,
                         scr16=scr16, etp=etp, outp=outp,
                         pp=pp, pav=pav, psums=psums, tc=tc)
            _body(nc, tc, pools,
                  xT_c, wlT_c, prior_b, negm_c, w_pair, a_src_p, a_dst_p,
                  outT, sums, sdst_dram)
    return nc


def _head_prep(nc, pools, h, hpT, w_pair, a_src_p, a_dst_p,
               sdst_dram, consts):
    """Per-head: tT, s2, d-cols + exps, srcb, E-rows, V."""
    headp, pp = pools["headp"], pools["pp"]
    ones_row, negm_cols = consts

    wh = headp.tile([128, 128], FP, tag="wh")
    nc.sync.dma_start(out=wh, in_=w_pair[h])
    acols = headp.tile([128, 2], FP, tag="acols")
    nc.sync.dma_start(out=acols[:, 0:1], in_=a_src_p[h][:, None])
    nc.sync.dma_start(out=acols[:, 1:2], in_=a_dst_p[h][:, None])
    acols_bf = headp.tile([128, 2], BF, tag="acols_bf")
    nc.vector.tensor_copy(acols_bf, acols)
    wh_r = headp.tile([128, 128], FR, tag="wh_r")
    nc.vector.tensor_copy(wh_r, wh)

    # ---- tT = tanh(wh.T @ hpT)  [128(p), M] bf16 ----
    tT = headp.tile([128, M], BF, tag="tT")
    for st, en in GRPS:
        ph = pp.tile([128, 512], FP, tag="tr")
        nc.tensor.matmul(ph[:, :en - st], wh_r, hpT[:, st:en],
                         start=True, stop=True)
        nc.scalar.activation(tT[:, st:en], ph[:, :en - st], Tanh)

    # ---- s2[0]=s_src, s2[1]=s_dst  [2, M] ----
    s2 = headp.tile([2, M], FR, tag="s2")
    for st, en in GRPS:
        ps2 = pp.tile([128, 512], FP, tag="tr")
        nc.tensor.matmul(ps2[:2, :en - st], acols_bf, tT[:, st:en],
                         start=True, stop=True)
        nc.vector.tensor_copy(s2[:, st:en], ps2[:2, :en - st])

    # ---- d_j as columns via DRAM bounce; fold pad mask; exp tables ----
    nc.sync.dma_start(out=sdst_dram[h, :], in_=s2[1:2, :].bitcast(FP))
    sdc = headp.tile([128, NCH], FP, tag="sdc")
    nc.sync.dma_start(out=sdc,
                      in_=sdst_dram[h, :].rearrange("(c p) -> p c", p=128))
    sdcm = headp.tile([128, NCH], FP, tag="sdcm")
    nc.vector.tensor_tensor(sdcm, sdc, negm_cols, op=ALU.add)
    sdc1 = headp.tile([128, NCH], FP, tag="sdc1")
    nc.vector.tensor_scalar_max(sdc1, sdcm, DCLAMP)
    sdc2 = headp.tile([128, NCH], FP, tag="sdc2")
    nc.vector.tensor_scalar(sdc2, sdcm, 0.2, DCLAMP, op0=ALU.mult, op1=ALU.max)
    f1c = headp.tile([128, NCH], FP, tag="f1c")
    nc.scalar.activation(f1c, sdc1, Exp)
    f2c = headp.tile([128, NCH], FP, tag="f2c")
    nc.scalar.activation(f2c, sdc2, Exp)

    # ---- srcb = broadcast of s_src over partitions; E rows ----
    srcb = headp.tile([128, M], FP, tag="srcb")
    E1rb = headp.tile([128, M], BF, tag="E1rb")
    E2rb = headp.tile([128, M], BF, tag="E2rb")
    for st, en in GRPS:
        pb = pp.tile([128, 512], FP, tag="tr")
        nc.tensor.matmul(pb[:, :en - st], ones_row, s2[0:1, st:en],
                         start=True, stop=True)
        nc.scalar.copy(srcb[:, st:en], pb[:, :en - st])
    nc.scalar.activation(E1rb, srcb, Exp)
    nc.scalar.activation(E2rb, srcb, Exp, scale=0.2)

    # ---- V = hp @ wh  [n(p), O] bf16, per 128-chunk ----
    V = headp.tile([128, M], BF, tag="V")
    for t in range(NCH):
        pv = pp.tile([128, 512], FP, tag="tr")
        nc.tensor.matmul(pv[:, :128], hpT[:, c128(t)], wh_r,
                         start=True, stop=True)
        if VCOPY[t] == "S":
            nc.scalar.copy(V[:, c128(t)], pv[:, :128])
        else:
            nc.vector.tensor_copy(V[:, c128(t)], pv[:, :128])

    return dict(tT=tT, s2=s2, sdcm=sdcm, sdc1=sdc1, sdc2=sdc2,
                f1c=f1c, f2c=f2c, srcb=srcb, E1rb=E1rb, E2rb=E2rb, V=V)


def _head_main(nc, pools, h, st, outT, sums, consts):
    scr16, etp = pools["scr16"], pools["etp"]
    headp, outp = pools["headp"], pools["outp"]
    pav, psums = pools["pav"], pools["psums"]
    ones_col_bf = consts

    srcb, sdc1, sdc2 = st["srcb"], st["sdc1"], st["sdc2"]
    E1rb, E2rb, f1c, f2c, V = st["E1rb"], st["E2rb"], st["f1c"], st["f2c"], st["V"]

    av = pav.tile([128, M], FP, tag="av")
    sump = psums.tile([65, 512], FP, tag="sump")

    def sum_slot(g, width):
        base = 32 * g
        return sump[base:base + 1, :width]

    for jc in range(NCH):
        route = ROUTES[jc]
        eT = etp.tile([128, M], BF, tag="eT")
        if route == "A":
            # e = max(exp(z), exp(0.2 z)) = exp(lrelu_0.2(z)), z = s_i + d_j
            t1 = scr16.tile([128, M], BF, tag="t1")
            nc.scalar.activation(t1, srcb, Exp, bias=sdc1[:, jc:jc + 1])
            t2 = scr16.tile([128, M], BF, tag="t2")
            nc.scalar.activation(t2, srcb, Exp, bias=sdc2[:, jc:jc + 1],
                                 scale=0.2)
        else:
            t1 = scr16.tile([128, M], BF, tag="t1")
            nc.vector.tensor_scalar(t1, E1rb, f1c[:, jc:jc + 1], None,
                                    op0=ALU.mult)
            t2 = scr16.tile([128, M], BF, tag="t2")
            nc.vector.tensor_scalar(t2, E2rb, f2c[:, jc:jc + 1], None,
                                    op0=ALU.mult)
        nc.vector.tensor_tensor(eT, t1, t2, op=ALU.max)
        for g, (gs, ge) in enumerate(GRPS):
            nc.tensor.matmul(av[:, gs:ge], V[:, c128(jc)], eT[:, gs:ge],
                             start=(jc == 0), stop=(jc == NCH - 1),
                             skip_group_check=True)
        for g, (gs, ge) in enumerate(GRPS):
            nc.tensor.matmul(sum_slot(g, ge - gs), ones_col_bf, eT[:, gs:ge],
                             start=(jc == 0), stop=(jc == NCH - 1),
                             skip_group_check=True)

    # ---- export unnormalized av + denominators; host divides ----
    sum_sb = headp.tile([1, M], BF, tag="sum_sb")
    for g, (gs, ge) in enumerate(GRPS):
        nc.vector.tensor_copy(sum_sb[:, gs:ge], sum_slot(g, ge - gs))
    nc.sync.dma_start(out=sums[h, :], in_=sum_sb)
    for g, (gs, ge) in enumerate(GRPS):
        outF = outp.tile([128, 512], BF, tag="outF")
        if g % 2 == 0:
            nc.scalar.copy(outF[:, :ge - gs], av[:, gs:ge])
        else:
            nc.vector.tensor_copy(outF[:, :ge - gs], av[:, gs:ge])
        nc.sync.dma_start(out=outT[h, :, gs:ge], in_=outF[:, :ge - gs])


def _body(nc, tc, pools,
          xT_c, wlT_c, prior_b, negm_c, w_pair, a_src_p, a_dst_p,
          outT, sums, sdst_dram):
    constp, bigp = pools["constp"], pools["bigp"]
    pp = pools["pp"]

    # ---- constants ----
    ones_row_f = constp.tile([1, 128], FP, tag="ones_row_f")
    nc.vector.memset(ones_row_f, 1.0)
    ones_row = constp.tile([1, 128], FR, tag="ones_row")
    nc.vector.tensor_copy(ones_row, ones_row_f)
    ones_col_bf = constp.tile([128, 1], BF, tag="ones_col_bf")
    nc.vector.memset(ones_col_bf, 1.0)
    negm_cols = constp.tile([128, NCH], FP, tag="negm_cols")
    nc.sync.dma_start(out=negm_cols, in_=negm_c[:, :])

    # ---- prep: hpT = (x_c @ W_lin.T).T from host-transposed bf16 inputs --
    hpT = bigp.tile([128, M], FR, tag="hpT")
    wlT = constp.tile([128, 2, 128], BF, tag="wlT")
    xT = bigp.tile([128, 2, M], BF, tag="xT")
    prior_sb = constp.tile([128, 1], FP, tag="prior_sb")
    nc.sync.dma_start(out=prior_sb, in_=prior_b[:, None])
    for k in range(2):
        nc.sync.dma_start(out=wlT[:, k, :], in_=wlT_c[k])
        nc.sync.dma_start(out=xT[:, k, :], in_=xT_c[k])
    for st, en in GRPS:
        ph = pp.tile([128, 512], FP, tag="tr")
        for k in range(2):
            nc.tensor.matmul(ph[:, :en - st], wlT[:, k, :], xT[:, k, st:en],
                             start=(k == 0), stop=(k == 1))
        nc.vector.tensor_copy(hpT[:, st:en], ph[:, :en - st])
    # slot 0 is reserved for the prior node
    nc.vector.tensor_copy(hpT[:, 0:1], prior_sb)

    consts_prep = (ones_row, negm_cols)
    sts = []
    for h in range(HPC):
        sts.append(_head_prep(nc, pools, h, hpT,
                              w_pair, a_src_p, a_dst_p,
                              sdst_dram, consts_prep))
    for h in range(HPC):
        _head_main(nc, pools, h, sts[h], outT, sums, ones_col_bf)


_NC_CACHE = None


def _get_nc():
    global _NC_CACHE
    if _NC_CACHE is None:
        nc = _build()
        nc.finalize()
        _NC_CACHE = nc
    return _NC_CACHE


def _compact(x, x_mask):
    """Per batch: slot 0 = prior node (2047), then unmasked nodes, then pads.

    Returns per-batch (xT_c bf16 [2,128,M], negm_c fp32 [M],
    idx array of real node ids for slots 1.., n_real, prior_keep).
    """
    import ml_dtypes
    B = x.shape[0]
    packs = []
    for b in range(B):
        keep = ~x_mask[b]
        others = np.nonzero(keep[:N])[0]
        n_real = 1 + len(others)
        assert n_real <= M, f"batch {b}: {n_real} unmasked nodes > M={M}"
        xc = np.zeros((M, I), np.float32)
        xc[1:n_real] = x[b][others]
        negm = np.zeros(M, np.float32)
        negm[n_real:] = NEG
        if not keep[N]:          # prior node masked -> slot 0 is a pad
            negm[0] = NEG
        negm = np.ascontiguousarray(negm.reshape(NCH, 128).T)
        xT = np.ascontiguousarray(
            xc.T.reshape(2, 128, M).astype(ml_dtypes.bfloat16))
        packs.append((xT, negm, others, n_real, bool(keep[N])))
    return packs


def make_in_maps(x, prior_feature, x_mask, W_lin, w_head, a_src, a_dst):
    import ml_dtypes
    packs = _compact(x, x_mask)
    wlT_c = np.ascontiguousarray(
        W_lin.T.reshape(2, 128, 128).astype(ml_dtypes.bfloat16))
    in_maps = []
    for c in range(NCORES):
        b, h0 = c // 2, (c % 2) * HPC
        xT, negm, _, _, _ = packs[b]
        in_maps.append(dict(
            xT_c=xT,
            wlT_c=wlT_c,
            prior_b=prior_feature[b],
            negm_c=negm,
            w_pair=np.ascontiguousarray(w_head[h0:h0 + HPC]),
            a_src_p=np.ascontiguousarray(a_src[h0:h0 + HPC]),
            a_dst_p=np.ascontiguousarray(a_dst[h0:h0 + HPC]),
        ))
    return packs, in_maps


def combine_results(results, packs, x, prior_feature, x_mask,
                    W_lin, w_head, bias):
    B = 4
    out = np.zeros((B, N1, O), np.float32)
    for c in range(NCORES):
        b = c // 2
        o = np.asarray(results[c]["outT"], np.float32)   # [HPC, O, M]
        s = np.asarray(results[c]["sums"], np.float32)    # [HPC, M]
        _, _, others, n_real, prior_keep = packs[b]
        contrib = ((o[0] / s[0][None, :] + o[1] / s[1][None, :]).T
                   * 0.25)[:n_real]
        if prior_keep:
            out[b, N] += contrib[0]
        out[b, others] += contrib[1:]
    # masked rows: exactly uniform attention = mean_j hp_h[j] (host, exact)
    xsum = x.sum(axis=1)                                   # [B, I]
    hp_mean = (xsum @ W_lin.T + prior_feature) / N1        # [B, O]
    vbar_sum = np.einsum('bo,hop->bp', hp_mean, w_head)    # sum over heads
    for b in range(B):
        out[b][x_mask[b], :] = 0.25 * vbar_sum[b][None, :]
    out += np.asarray(bias, np.float32)[None, None, :]
    return out


def kernel(x, prior_feature, x_mask, W_lin, w_head, a_src, a_dst, bias,
           **run_kwargs):
    from concourse.bass_utils import run_bass_kernel_spmd
    nc = _get_nc()
    x = np.ascontiguousarray(np.asarray(x, np.float32))
    prior_feature = np.ascontiguousarray(np.asarray(prior_feature, np.float32))
    x_mask = np.asarray(x_mask, bool)
    W_lin = np.ascontiguousarray(np.asarray(W_lin, np.float32))
    w_head = np.ascontiguousarray(np.asarray(w_head, np.float32))
    a_src = np.ascontiguousarray(np.asarray(a_src, np.float32))
    a_dst = np.ascontiguousarray(np.asarray(a_dst, np.float32))
    packs, in_maps = make_in_maps(x, prior_feature, x_mask, W_lin, w_head,
                                  a_src, a_dst)
    br = run_bass_kernel_spmd(nc, in_maps, core_ids=list(range(NCORES)),
                              **run_kwargs)
    out = combine_results(br.results, packs, x, prior_feature, x_mask,
                          W_lin, w_head, bias)
    if run_kwargs:
        kernel.last_bass_results = br
    return out


# revision 19
# speedup vs baseline: 1.1819x; 1.0527x over previous
"""GAT layer kernel for Trainium2, SPMD over 8 NeuronCores.

Reference computation (per batch b):
  h  = x @ W_lin.T                          [N, O]
  hp = concat(h, prior[None, :])            [N1, O]
  per head: hp_h = hp @ w_head[h]           [N1, O]
  t = tanh(hp_h); s_src = t @ a_src[h]; s_dst = t @ a_dst[h]
  z[i,j] = s_src[i] + s_dst[j]; y = leaky_relu(z, 0.2)
  y[mask_i | mask_j] = -1e18; p = softmax_j(y)
  out_h = p @ hp_h;  out = mean_h(out_h) + bias

Sharding: core c handles batch b=c//2 and heads h in {2*(c%2), 2*(c%2)+1}.

Mask-compaction: masked-j columns get zero attention weight, and masked-i
rows are exactly uniform attention (handled on host via the head's mean
value row vbar, computed on host -- it is linear in the inputs).  So the
device only processes the ~1000 UNMASKED nodes per batch: the host
compacts x to M=1280 padded slots (slot 0 reserved for the prior node,
tail slots padded; pads are forced to zero weight via a -400 sentinel
folded into their d_j), pre-transposes x and W_lin (bf16 -- the PE's
float32r mode rounds operands to bf16 anyway), and scatters the result
back to full [N1, O].  This shrinks the e-matrix work ~4x.

Per core and head the kernel computes the transposed partial output
  outT[h] = sum_j hp_h[j,:] * e[j,i]   in [O, M]    (unnormalized)
and the softmax denominators sums[h][M]; the host divides, scatters,
fixes masked rows with vbar, averages heads, adds bias.

e is generated by two engine routes (tunable per j-chunk), using
exp(lrelu(z)) = max(exp(z), exp(0.2 z)):
  A (ACT):  e1 = Exp(s + d'[j]-bias), e2 = Exp(0.2 s + 0.2 d''[j])
  V (DVE):  rank-1 t1 = E1*f1[j], t2 = E2*f2[j]  (exp(s_i+d_j) =
            exp(s_i)*exp(d_j)); E-rows precomputed once per head
+ a shared DVE tensor_tensor max.  Row-side (i) rounding cancels exactly
in the softmax; only the j side needs fp32-accurate exponents.  e and V
are bf16 so the dominant PE streams run at 1 cycle/column.
"""

import sys

for _p in ("/opt/trn_rl_repo",):
    if _p not in sys.path:
        sys.path.insert(0, _p)

import os as _os

import numpy as np

import concourse.bass as bass
import concourse.tile as tile
from concourse import bacc, mybir

FP = mybir.dt.float32
FR = mybir.dt.float32r
BF = mybir.dt.bfloat16
U8 = mybir.dt.uint8
N, N1, I, O = 2047, 2048, 256, 128
M = 1152          # compacted node slots (>= max unmasked count, 9*128)
NCH = M // 128    # j-chunks
GRPS = [(0, 512), (512, 1024), (1024, M)]  # i-column groups (PSUM banks)
HPC = 2  # heads per core
NCORES = 8
NEG = -400.0    # pad sentinel folded into d_j
DCLAMP = -43.0  # keeps every exp input inside the ACT table (~[-87, 88])
Tanh = mybir.ActivationFunctionType.Tanh
Exp = mybir.ActivationFunctionType.Exp
ALU = mybir.AluOpType

# per-jc e-generation route, A=ACT-heavy, V=DVE rank-1 (see module doc)
ROUTES = _os.environ.get("GAT_ROUTES", "AVVAVAVVA")
assert len(ROUTES) == NCH and set(ROUTES) <= set("AV")
# engine for the per-head V=hp@wh PSUM->SBUF casts (gpsimd cannot read PSUM)
VCOPY = _os.environ.get("GAT_VCOPY", "SVSVSVSVS")
assert len(VCOPY) == NCH and set(VCOPY) <= set("SV")


def c128(c):
    return slice(c * 128, (c + 1) * 128)


def _build() -> bass.Bass:
    nc = bacc.Bacc(None, target_bir_lowering=False, debug=False)
    xT_c = nc.dram_tensor("xT_c", [2, 128, M], BF, kind="ExternalInput")
    wlT_c = nc.dram_tensor("wlT_c", [2, 128, 128], BF, kind="ExternalInput")
    prior_b = nc.dram_tensor("prior_b", [O], FP, kind="ExternalInput")
    negm_c = nc.dram_tensor("negm_c", [128, NCH], FP, kind="ExternalInput")
    w_pair = nc.dram_tensor("w_pair", [HPC, O, O], FP, kind="ExternalInput")
    a_src_p = nc.dram_tensor("a_src_p", [HPC, O], FP, kind="ExternalInput")
    a_dst_p = nc.dram_tensor("a_dst_p", [HPC, O], FP, kind="ExternalInput")
    outT = nc.dram_tensor("outT", [HPC, O, M], BF, kind="ExternalOutput")
    sums = nc.dram_tensor("sums", [HPC, M], BF, kind="ExternalOutput")
    sdst_dram = nc.dram_tensor("sdst_scratch", [HPC, M], FP)

    with tile.TileContext(nc) as tc:
        with (
            tc.tile_pool(name="constp", bufs=1) as constp,
            tc.tile_pool(name="bigp", bufs=1) as bigp,
            tc.tile_pool(name="headp", bufs=2) as headp,
            tc.tile_pool(name="scr16", bufs=6) as scr16,
            tc.tile_pool(name="etp", bufs=8) as etp,
            tc.tile_pool(name="outp", bufs=4) as outp,
            tc.tile_pool(name="pp", bufs=3, space="PSUM") as pp,
            tc.tile_pool(name="pav", bufs=1, space="PSUM") as pav,
            tc.tile_pool(name="psums", bufs=1, space="PSUM") as psums,
        ):
            pools = dict(constp=constp, bigp=bigp, headp=headp,
                         scr16=scr16, etp=etp, outp=outp,
                         pp=pp, pav=pav, psums=psums, tc=tc)
            _body(nc, tc, pools,
                  xT_c, wlT_c, prior_b, negm_c, w_pair, a_src_p, a_dst_p,
                  outT, sums, sdst_dram)
    return nc


def _head_prep(nc, pools, h, hpT, w_pair, a_src_p, a_dst_p,
               sdst_dram, consts):
    """Per-head: tT, s2, d-cols + exps, srcb, E-rows, V."""
    headp, pp = pools["headp"], pools["pp"]
    ones_row, negm_cols = consts

    wh = headp.tile([128, 128], FP, tag="wh")
    nc.sync.dma_start(out=wh, in_=w_pair[h])
    acols = headp.tile([128, 2], FP, tag="acols")
    nc.sync.dma_start(out=acols[:, 0:1], in_=a_src_p[h][:, None])
    nc.sync.dma_start(out=acols[:, 1:2], in_=a_dst_p[h][:, None])
    acols_bf = headp.tile([128, 2], BF, tag="acols_bf")
    nc.vector.tensor_copy(acols_bf, acols)
    wh_r = headp.tile([128, 128], FR, tag="wh_r")
    nc.vector.tensor_copy(wh_r, wh)

    # ---- tT = tanh(wh.T @ hpT)  [128(p), M] bf16 ----
    tT = headp.tile([128, M], BF, tag="tT")
    for st, en in GRPS:
        ph = pp.tile([128, 512], FP, tag="tr")
        nc.tensor.matmul(ph[:, :en - st], wh_r, hpT[:, st:en],
                         start=True, stop=True)
        nc.scalar.activation(tT[:, st:en], ph[:, :en - st], Tanh)

    # ---- s2[0]=s_src, s2[1]=s_dst  [2, M] ----
    s2 = headp.tile([2, M], FR, tag="s2")
    for st, en in GRPS:
        ps2 = pp.tile([128, 512], FP, tag="tr")
        nc.tensor.matmul(ps2[:2, :en - st], acols_bf, tT[:, st:en],
                         start=True, stop=True)
        nc.vector.tensor_copy(s2[:, st:en], ps2[:2, :en - st])

    # ---- d_j as columns via DRAM bounce; fold pad mask; exp tables ----
    nc.sync.dma_start(out=sdst_dram[h, :], in_=s2[1:2, :].bitcast(FP))
    sdc = headp.tile([128, NCH], FP, tag="sdc")
    nc.sync.dma_start(out=sdc,
                      in_=sdst_dram[h, :].rearrange("(c p) -> p c", p=128))
    sdcm = headp.tile([128, NCH], FP, tag="sdcm")
    nc.vector.tensor_tensor(sdcm, sdc, negm_cols, op=ALU.add)
    sdc1 = headp.tile([128, NCH], FP, tag="sdc1")
    nc.vector.tensor_scalar_max(sdc1, sdcm, DCLAMP)
    sdc2 = headp.tile([128, NCH], FP, tag="sdc2")
    nc.vector.tensor_scalar(sdc2, sdcm, 0.2, DCLAMP, op0=ALU.mult, op1=ALU.max)
    f1c = headp.tile([128, NCH], FP, tag="f1c")
    nc.scalar.activation(f1c, sdc1, Exp)
    f2c = headp.tile([128, NCH], FP, tag="f2c")
    nc.scalar.activation(f2c, sdc2, Exp)

    # ---- srcb = broadcast of s_src over partitions; E rows ----
    srcb = headp.tile([128, M], FP, tag="srcb")
    E1rb = headp.tile([128, M], BF, tag="E1rb")
    E2rb = headp.tile([128, M], BF, tag="E2rb")
    for st, en in GRPS:
        pb = pp.tile([128, 512], FP, tag="tr")
        nc.tensor.matmul(pb[:, :en - st], ones_row, s2[0:1, st:en],
                         start=True, stop=True)
        nc.scalar.copy(srcb[:, st:en], pb[:, :en - st])
    nc.scalar.activation(E1rb, srcb, Exp)
    nc.scalar.activation(E2rb, srcb, Exp, scale=0.2)

    # ---- V = hp @ wh  [n(p), O] bf16, per 128-chunk ----
    V = headp.tile([128, M], BF, tag="V")
    for t in range(NCH):
        pv = pp.tile([128, 512], FP, tag="tr")
        nc.tensor.matmul(pv[:, :128], hpT[:, c128(t)], wh_r,
                         start=True, stop=True)
        if VCOPY[t] == "S":
            nc.scalar.copy(V[:, c128(t)], pv[:, :128])
        else:
            nc.vector.tensor_copy(V[:, c128(t)], pv[:, :128])

    return dict(tT=tT, s2=s2, sdcm=sdcm, sdc1=sdc1, sdc2=sdc2,
                f1c=f1c, f2c=f2c, srcb=srcb, E1rb=E1rb, E2rb=E2rb, V=V)


def _head_main(nc, pools, h, st, outT, sums, consts):
    scr16, etp = pools["scr16"], pools["etp"]
    headp, outp = pools["headp"], pools["outp"]
    pav, psums = pools["pav"], pools["psums"]
    ones_col_bf = consts

    srcb, sdc1, sdc2 = st["srcb"], st["sdc1"], st["sdc2"]
    E1rb, E2rb, f1c, f2c, V = st["E1rb"], st["E2rb"], st["f1c"], st["f2c"], st["V"]

    av = pav.tile([128, M], FP, tag="av")
    sump = psums.tile([65, 512], FP, tag="sump")

    def sum_slot(g, width):
        base = 32 * g
        return sump[base:base + 1, :width]

    for jc in range(NCH):
        route = ROUTES[jc]
        eT = etp.tile([128, M], BF, tag="eT")
        if route == "A":
            # e = max(exp(z), exp(0.2 z)) = exp(lrelu_0.2(z)), z = s_i + d_j
            t1 = scr16.tile([128, M], BF, tag="t1")
            nc.scalar.activation(t1, srcb, Exp, bias=sdc1[:, jc:jc + 1])
            t2 = scr16.tile([128, M], BF, tag="t2")
            nc.scalar.activation(t2, srcb, Exp, bias=sdc2[:, jc:jc + 1],
                                 scale=0.2)
        else:
            t1 = scr16.tile([128, M], BF, tag="t1")
            nc.vector.tensor_scalar(t1, E1rb, f1c[:, jc:jc + 1], None,
                                    op0=ALU.mult)
            t2 = scr16.tile([128, M], BF, tag="t2")
            nc.vector.tensor_scalar(t2, E2rb, f2c[:, jc:jc + 1], None,
                                    op0=ALU.mult)
        nc.vector.tensor_tensor(eT, t1, t2, op=ALU.max)
        for g, (gs, ge) in enumerate(GRPS):
            nc.tensor.matmul(av[:, gs:ge], V[:, c128(jc)], eT[:, gs:ge],
                             start=(jc == 0), stop=(jc == NCH - 1),
                             skip_group_check=True)
        for g, (gs, ge) in enumerate(GRPS):
            nc.tensor.matmul(sum_slot(g, ge - gs), ones_col_bf, eT[:, gs:ge],
                             start=(jc == 0), stop=(jc == NCH - 1),
                             skip_group_check=True)

    # ---- export unnormalized av + denominators; host divides ----
    sum_sb = headp.tile([1, M], BF, tag="sum_sb")
    for g, (gs, ge) in enumerate(GRPS):
        nc.vector.tensor_copy(sum_sb[:, gs:ge], sum_slot(g, ge - gs))
    nc.sync.dma_start(out=sums[h, :], in_=sum_sb)
    dma_eng = [nc.sync, nc.scalar, nc.gpsimd]
    for g, (gs, ge) in enumerate(GRPS):
        outF = outp.tile([128, 512], BF, tag="outF")
        if g % 2 == 0:
            nc.scalar.copy(outF[:, :ge - gs], av[:, gs:ge])
        else:
            nc.vector.tensor_copy(outF[:, :ge - gs], av[:, gs:ge])
        dma_eng[g].dma_start(out=outT[h, :, gs:ge], in_=outF[:, :ge - gs])


def _body(nc, tc, pools,
          xT_c, wlT_c, prior_b, negm_c, w_pair, a_src_p, a_dst_p,
          outT, sums, sdst_dram):
    constp, bigp = pools["constp"], pools["bigp"]
    pp = pools["pp"]

    # ---- constants ----
    ones_row_f = constp.tile([1, 128], FP, tag="ones_row_f")
    nc.vector.memset(ones_row_f, 1.0)
    ones_row = constp.tile([1, 128], FR, tag="ones_row")
    nc.vector.tensor_copy(ones_row, ones_row_f)
    ones_col_bf = constp.tile([128, 1], BF, tag="ones_col_bf")
    nc.vector.memset(ones_col_bf, 1.0)
    negm_cols = constp.tile([128, NCH], FP, tag="negm_cols")
    nc.sync.dma_start(out=negm_cols, in_=negm_c[:, :])

    # ---- prep: hpT = (x_c @ W_lin.T).T from host-transposed bf16 inputs --
    hpT = bigp.tile([128, M], FR, tag="hpT")
    wlT = constp.tile([128, 2, 128], BF, tag="wlT")
    xT = bigp.tile([128, 2, M], BF, tag="xT")
    prior_sb = constp.tile([128, 1], FP, tag="prior_sb")
    nc.sync.dma_start(out=prior_sb, in_=prior_b[:, None])
    for k in range(2):
        nc.sync.dma_start(out=wlT[:, k, :], in_=wlT_c[k])
        nc.sync.dma_start(out=xT[:, k, :], in_=xT_c[k])
    for st, en in GRPS:
        ph = pp.tile([128, 512], FP, tag="tr")
        for k in range(2):
            nc.tensor.matmul(ph[:, :en - st], wlT[:, k, :], xT[:, k, st:en],
                             start=(k == 0), stop=(k == 1))
        nc.vector.tensor_copy(hpT[:, st:en], ph[:, :en - st])
    # slot 0 is reserved for the prior node
    nc.vector.tensor_copy(hpT[:, 0:1], prior_sb)

    consts_prep = (ones_row, negm_cols)
    sts = []
    for h in range(HPC):
        sts.append(_head_prep(nc, pools, h, hpT,
                              w_pair, a_src_p, a_dst_p,
                              sdst_dram, consts_prep))
    for h in range(HPC):
        _head_main(nc, pools, h, sts[h], outT, sums, ones_col_bf)


_NC_CACHE = None


def _get_nc():
    global _NC_CACHE
    if _NC_CACHE is None:
        nc = _build()
        nc.finalize()
        _NC_CACHE = nc
    return _NC_CACHE


def _compact(x, x_mask):
    """Per batch: slot 0 = prior node (2047), then unmasked nodes, then pads.

    Returns per-batch (xT_c bf16 [2,128,M], negm_c fp32 [M],
    idx array of real node ids for slots 1.., n_real, prior_keep).
    """
    import ml_dtypes
    B = x.shape[0]
    packs = []
    for b in range(B):
        keep = ~x_mask[b]
        others = np.nonzero(keep[:N])[0]
        n_real = 1 + len(others)
        assert n_real <= M, f"batch {b}: {n_real} unmasked nodes > M={M}"
        xc = np.zeros((M, I), np.float32)
        xc[1:n_real] = x[b][others]
        negm = np.zeros(M, np.float32)
        negm[n_real:] = NEG
        if not keep[N]:          # prior node masked -> slot 0 is a pad
            negm[0] = NEG
        negm = np.ascontiguousarray(negm.reshape(NCH, 128).T)
        xT = np.ascontiguousarray(
            xc.T.reshape(2, 128, M).astype(ml_dtypes.bfloat16))
        packs.append((xT, negm, others, n_real, bool(keep[N])))
    return packs


def make_in_maps(x, prior_feature, x_mask, W_lin, w_head, a_src, a_dst):
    import ml_dtypes
    packs = _compact(x, x_mask)
    wlT_c = np.ascontiguousarray(
        W_lin.T.reshape(2, 128, 128).astype(ml_dtypes.bfloat16))
    in_maps = []
    for c in range(NCORES):
        b, h0 = c // 2, (c % 2) * HPC
        xT, negm, _, _, _ = packs[b]
        in_maps.append(dict(
            xT_c=xT,
            wlT_c=wlT_c,
            prior_b=prior_feature[b],
            negm_c=negm,
            w_pair=np.ascontiguousarray(w_head[h0:h0 + HPC]),
            a_src_p=np.ascontiguousarray(a_src[h0:h0 + HPC]),
            a_dst_p=np.ascontiguousarray(a_dst[h0:h0 + HPC]),
        ))
    return packs, in_maps


def combine_results(results, packs, x, prior_feature, x_mask,
                    W_lin, w_head, bias):
    B = 4
    out = np.zeros((B, N1, O), np.float32)
    for c in range(NCORES):
        b = c // 2
        o = np.asarray(results[c]["outT"], np.float32)   # [HPC, O, M]
        s = np.asarray(results[c]["sums"], np.float32)    # [HPC, M]
        _, _, others, n_real, prior_keep = packs[b]
        contrib = ((o[0] / s[0][None, :] + o[1] / s[1][None, :]).T
                   * 0.25)[:n_real]
        if prior_keep:
            out[b, N] += contrib[0]
        out[b, others] += contrib[1:]
    # masked rows: exactly uniform attention = mean_j hp_h[j] (host, exact)
    xsum = x.sum(axis=1)                                   # [B, I]
    hp_mean = (xsum @ W_lin.T + prior_feature) / N1        # [B, O]
    vbar_sum = np.einsum('bo,hop->bp', hp_mean, w_head)    # sum over heads
    for b in range(B):
        out[b][x_mask[b], :] = 0.25 * vbar_sum[b][None, :]
    out += np.asarray(bias, np.float32)[None, None, :]
    return out


def kernel(x, prior_feature, x_mask, W_lin, w_head, a_src, a_dst, bias,
           **run_kwargs):
    from concourse.bass_utils import run_bass_kernel_spmd
    nc = _get_nc()
    x = np.ascontiguousarray(np.asarray(x, np.float32))
    prior_feature = np.ascontiguousarray(np.asarray(prior_feature, np.float32))
    x_mask = np.asarray(x_mask, bool)
    W_lin = np.ascontiguousarray(np.asarray(W_lin, np.float32))
    w_head = np.ascontiguousarray(np.asarray(w_head, np.float32))
    a_src = np.ascontiguousarray(np.asarray(a_src, np.float32))
    a_dst = np.ascontiguousarray(np.asarray(a_dst, np.float32))
    packs, in_maps = make_in_maps(x, prior_feature, x_mask, W_lin, w_head,
                                  a_src, a_dst)
    br = run_bass_kernel_spmd(nc, in_maps, core_ids=list(range(NCORES)),
                              **run_kwargs)
    out = combine_results(br.results, packs, x, prior_feature, x_mask,
                          W_lin, w_head, bias)
    if run_kwargs:
        kernel.last_bass_results = br
    return out
